# revision 1
# baseline (speedup 1.0000x reference)
"""Multi-head attention Trainium2 kernel (bs=4, slen=1024, dim=1024, 16 heads).

Sharding: 8 cores = 4 batches x 2 head-groups (8 heads / 512 features each).
Per core (batch b, head-group g):
  qT/kT projections in [feature, seq] layout, v in [seq, feature] layout,
  scoresT = kT.T @ qT per head in [key, query] layout (dh=64 contraction,
  2 heads in the two 64-partition halves),
  exp on ScalarE with the key-padding mask folded into the per-partition bias,
  ctxT = v_aug.T @ expT accumulated over key tiles, an all-ones column
  appended to v making row 64 of ctxT the softmax denominator,
  ctxT normalization by broadcasting 1/denominator with a K=1 matmul,
  partial out-projection [seq, dim] for this head-group.
Host sums the two head-group partials per batch and adds out_b.

Matmul operand dtypes are configurable per stage: float32r (tf32-like fast
fp32 path, ~2 cyc/col) or bfloat16 (~1 cyc/col, lower precision).
"""

import numpy as np

BS, SLEN, DIM = 4, 1024, 1024
H, DH = 16, 64
P = 128            # partitions
NB = 512           # matmul free-dim chunk (one PSUM bank of fp32)
FPC = 512          # features per core (8 heads)
DT = DIM // P      # 8 contraction tiles over model dim
FT = FPC // P      # 4 feature tiles per core
QC = SLEN // NB    # 2 seq chunks
ST = SLEN // P     # 8 seq tiles
HP = 4             # head pairs per core

# matmul operand dtypes per stage: "f32r" or "bf16"
PROJ_DT = "f32r"
ATT_DT = "f32r"
OUT_DT = "f32r"

_STATE = {}

# set to True by test harness to capture an NTFF profile
TRACE = False
TRACE_KWARGS = {}
LAST_RESULT = None


def _np_dt(kind):
    if kind == "bf16":
        import ml_dtypes
        return ml_dtypes.bfloat16
    return np.float32


def _build():
    from contextlib import ExitStack

    import concourse.tile as tile
    from concourse import bacc, mybir

    f32 = mybir.dt.float32
    f32r = mybir.dt.float32r
    bf16 = mybir.dt.bfloat16
    AF = mybir.ActivationFunctionType

    dts = {"f32r": f32r, "bf16": bf16}
    pdt, adt, odt = dts[PROJ_DT], dts[ATT_DT], dts[OUT_DT]

    nc = bacc.Bacc("TRN2", target_bir_lowering=False, debug=False)

    xt_d = nc.dram_tensor("xt", [DIM, SLEN], pdt, kind="ExternalInput")
    wqt_d = nc.dram_tensor("wqt", [DIM, FPC], pdt, kind="ExternalInput")
    wkt_d = nc.dram_tensor("wkt", [DIM, FPC], pdt, kind="ExternalInput")
    wvt_d = nc.dram_tensor("wvt", [DIM, FPC], pdt, kind="ExternalInput")
    wot_d = nc.dram_tensor("wot", [FPC, DIM], odt, kind="ExternalInput")
    qb_d = nc.dram_tensor("qb", [P, FT], f32, kind="ExternalInput")
    kb_d = nc.dram_tensor("kb", [P, FT], f32, kind="ExternalInput")
    vb_d = nc.dram_tensor("vb", [1, FPC], pdt, kind="ExternalInput")
    negb_d = nc.dram_tensor("negb", [P, ST], f32, kind="ExternalInput")
    sel_d = nc.dram_tensor("sel", [2 * HP, HP, P], f32r, kind="ExternalInput")
    out_d = nc.dram_tensor("out", [SLEN, DIM], f32, kind="ExternalOutput")

    with tile.TileContext(nc) as tc:
        with ExitStack() as ctx:
            consts = ctx.enter_context(tc.tile_pool(name="consts", bufs=1))
            big = ctx.enter_context(tc.tile_pool(name="big", bufs=7))
            io = ctx.enter_context(tc.tile_pool(name="io", bufs=1))
            sm = ctx.enter_context(tc.tile_pool(name="sm", bufs=3))
            psum = ctx.enter_context(tc.tile_pool(name="psum", bufs=1, space="PSUM"))

            # ---- constants / small inputs ----
            qb_sb = consts.tile([P, FT], f32)
            nc.sync.dma_start(qb_sb, qb_d[:])
            kb_sb = consts.tile([P, FT], f32)
            nc.sync.dma_start(kb_sb, kb_d[:])
            vb_sb = consts.tile([1, FPC], pdt)
            nc.sync.dma_start(vb_sb, vb_d[:])
            negb_sb = consts.tile([P, ST], f32)
            nc.sync.dma_start(negb_sb, negb_d[:])
            sel_sb = consts.tile([2 * HP, HP, P], f32r)
            nc.sync.dma_start(sel_sb, sel_d[:])
            ones_f = consts.tile([1, P], f32)
            nc.vector.memset(ones_f, 1.0)
            ones_p = consts.tile([1, P], pdt)   # v-bias row lhsT
            nc.vector.tensor_copy(ones_p, ones_f)
            ones_r = consts.tile([1, P], f32r)  # denominator-broadcast lhsT
            nc.vector.tensor_copy(ones_r, ones_f)

            # ---- weight / activation inputs ----
            wqt_sb = big.tile([P, DT, FPC], pdt, tag="big")
            wkt_sb = big.tile([P, DT, FPC], pdt, tag="big")
            wvt_sb = big.tile([P, DT, FPC], pdt, tag="big")
            xt_a = big.tile([P, DT // 2, SLEN], pdt, tag="big")
            xt_b = big.tile([P, DT // 2, SLEN], pdt, tag="big")
            for t in range(DT):
                nc.sync.dma_start(
                    wqt_sb[:, t, :], wqt_d[t * P:(t + 1) * P, :])
                xh = xt_a if t < DT // 2 else xt_b
                nc.sync.dma_start(
                    xh[:, t % (DT // 2), :], xt_d[t * P:(t + 1) * P, :])
            for t in range(DT):
                nc.sync.dma_start(
                    wkt_sb[:, t, :], wkt_d[t * P:(t + 1) * P, :])
            for t in range(DT):
                nc.sync.dma_start(
                    wvt_sb[:, t, :], wvt_d[t * P:(t + 1) * P, :])

            def xts(t):
                xh = xt_a if t < DT // 2 else xt_b
                return xh[:, t % (DT // 2), :]

            # ---- projection outputs ----
            qT_sb = big.tile([P, FT, SLEN], adt, tag="big")  # [f%128, ft, seq]
            kT_sb = big.tile([P, FT, SLEN], adt, tag="big")
            v_sb = io.tile([P, ST, HP * 2, DH + 1], adt)  # [seq%128, st, h, e]
            ctxn_sb = io.tile([P, HP, SLEN], odt)  # normalized ctx.T per pair
            ctall = io.tile([P, HP * QC, NB], f32)   # unnormalized ctxT pairs
            rca = [io.tile([2 * HP, NB], f32r, name=f"rca{q}") for q in range(QC)]
            dnl = [io.tile([2 * HP, NB], f32, name=f"dnl{q}") for q in range(QC)]
            vones_f = consts.tile([P, ST, HP * 2, 1], f32)
            nc.vector.memset(vones_f, 1.0)
            nc.vector.tensor_copy(v_sb[:, :, :, DH:DH + 1], vones_f)

            # ---- phase 1: q/k projections (feature-major) ----
            with nc.named_scope("proj_qk"):
                for ft in range(FT):
                    for qc in range(QC):
                        sl = slice(qc * NB, (qc + 1) * NB)
                        ps_q = psum.tile([P, NB], f32, tag="mm", bufs=4)
                        for t in range(DT):
                            nc.tensor.matmul(
                                ps_q,
                                lhsT=wqt_sb[:, t, ft * P:(ft + 1) * P],
                                rhs=xts(t)[:, sl],
                                start=(t == 0), stop=(t == DT - 1))
                        nc.scalar.activation(
                            qT_sb[:, ft, sl], ps_q, AF.Identity,
                            bias=qb_sb[:, ft:ft + 1], scale=0.125)
                        ps_k = psum.tile([P, NB], f32, tag="mm", bufs=4)
                        for t in range(DT):
                            nc.tensor.matmul(
                                ps_k,
                                lhsT=wkt_sb[:, t, ft * P:(ft + 1) * P],
                                rhs=xts(t)[:, sl],
                                start=(t == 0), stop=(t == DT - 1))
                        nc.scalar.activation(
                            kT_sb[:, ft, sl], ps_k, AF.Identity,
                            bias=kb_sb[:, ft:ft + 1], scale=1.0)

            # ---- phase 1b: v projection (seq-major) + bias via ones-row ----
            with nc.named_scope("proj_v"):
                for st in range(ST):
                    ps_v = psum.tile([P, NB], f32, tag="mm", bufs=4)
                    for t in range(DT):
                        nc.tensor.matmul(
                            ps_v,
                            lhsT=xts(t)[:, st * P:(st + 1) * P],
                            rhs=wvt_sb[:, t, :],
                            start=(t == 0), stop=False)
                    nc.tensor.matmul(
                        ps_v, lhsT=ones_p[0:1, 0:P], rhs=vb_sb[0:1, :],
                        start=False, stop=True)
                    nc.vector.tensor_copy(
                        v_sb[:, st, :, 0:DH],
                        ps_v.rearrange("p (h e) -> p h e", h=HP * 2))

            # out-projection weights (loaded into a later-freed big slot)
            wot_sb = big.tile([P, FT, DIM], odt, tag="big")
            for t in range(FT):
                nc.sync.dma_start(wot_sb[:, t, :], wot_d[t * P:(t + 1) * P, :])

            # ---- phase 2: attention, qc-outer; normalize + out-proj per qc ----
            for qc in range(QC):
                sl = slice(qc * NB, (qc + 1) * NB)
                with nc.named_scope("attn"):
                    for hp in range(HP):
                        wtsA = big.tile([P, ST, NB], adt, tag="big", name="wtsA")
                        wtsB = big.tile([P, ST, NB], adt, tag="big", name="wtsB")
                        # scoresT = kT.T @ qT (two heads in partition halves)
                        for kt in range(ST):
                            ksl = slice(kt * P, (kt + 1) * P)
                            psA = psum.tile([P, NB], f32, tag="mm", bufs=4,
                                            name="psA")
                            psB = psum.tile([P, NB], f32, tag="mm", bufs=4,
                                            name="psB")
                            nc.tensor.matmul(
                                psA,
                                lhsT=kT_sb[0:DH, hp, ksl],
                                rhs=qT_sb[0:DH, hp, sl],
                                tile_position=(0, 0))
                            nc.tensor.matmul(
                                psB,
                                lhsT=kT_sb[DH:P, hp, ksl],
                                rhs=qT_sb[DH:P, hp, sl],
                                tile_position=(DH, 0))
                            nc.scalar.activation(
                                wtsA[:, kt, :], psA, AF.Exp,
                                bias=negb_sb[:, kt:kt + 1], scale=1.0)
                            nc.scalar.activation(
                                wtsB[:, kt, :], psB, AF.Exp,
                                bias=negb_sb[:, kt:kt + 1], scale=1.0)
                        # ctxT (+denominator row) accumulated over key tiles
                        pcA = psum.tile([DH + 1, NB], f32, tag="ctx", bufs=2,
                                        name="pcA")
                        pcB = psum.tile([DH + 1, NB], f32, tag="ctx", bufs=2,
                                        name="pcB")
                        for kt in range(ST):
                            nc.tensor.matmul(
                                pcA, lhsT=v_sb[:, kt, 2 * hp, :],
                                rhs=wtsA[:, kt, :],
                                start=(kt == 0), stop=(kt == ST - 1))
                            nc.tensor.matmul(
                                pcB, lhsT=v_sb[:, kt, 2 * hp + 1, :],
                                rhs=wtsB[:, kt, :],
                                start=(kt == 0), stop=(kt == ST - 1))
                        # stash ctxT + denominator row (normalize deferred)
                        j = hp * QC + qc
                        for a, pc in ((0, pcA), (1, pcB)):
                            dtmp = sm.tile([1, NB], f32, tag="dtmp", bufs=4,
                                           name="dtmp")
                            nc.scalar.copy(dtmp, pc[DH:DH + 1, :])
                            nc.sync.dma_start(
                                dnl[qc][2 * hp + a:2 * hp + a + 1, :], dtmp)
                            nc.scalar.copy(
                                ctall[a * DH:(a + 1) * DH, j, :], pc[0:DH, :])

                # normalize this qc: ctxn = ctxT * bcast(1/denom) via selector
                with nc.named_scope("normalize"):
                    with nc.allow_low_precision(reason="softmax recip"):
                        nc.vector.reciprocal(rca[qc][:], dnl[qc][:])
                    for hp in range(HP):
                        j = hp * QC + qc
                        pb = psum.tile([P, NB], f32, tag="bc", bufs=2,
                                       name="pb")
                        nc.tensor.matmul(
                            pb, lhsT=sel_sb[:, hp, :], rhs=rca[qc][:])
                        nc.vector.tensor_mul(
                            ctxn_sb[:, hp, sl], ctall[:, j, :], pb)

                # partial out-projection for this qc's seq tiles
                with nc.named_scope("outproj"):
                    for qt in range(qc * ST // QC, (qc + 1) * ST // QC):
                        for jc in range(QC):
                            po = psum.tile([P, NB], f32, tag="mm", bufs=4,
                                           name="po")
                            for ft in range(FT):
                                nc.tensor.matmul(
                                    po,
                                    lhsT=ctxn_sb[:, ft, qt * P:(qt + 1) * P],
                                    rhs=wot_sb[:, ft, jc * NB:(jc + 1) * NB],
                                    start=(ft == 0), stop=(ft == FT - 1))
                            ob = sm.tile([P, NB], f32, tag="outsb", bufs=4,
                                         name="ob")
                            nc.vector.tensor_copy(ob, po)
                            nc.sync.dma_start(
                                out_d[qt * P:(qt + 1) * P,
                                      jc * NB:(jc + 1) * NB],
                                ob)

    nc.compile()
    return nc


def _get_nc():
    if "nc" not in _STATE:
        _STATE["nc"] = _build()
    return _STATE["nc"]


def _sel_const():
    sel = np.zeros((2 * HP, HP, P), np.float32)
    for hp in range(HP):
        sel[2 * hp, hp, 0:DH] = 1.0
        sel[2 * hp + 1, hp, DH:P] = 1.0
    return sel


def _in_maps(x, mask, q_w, q_b, k_w, k_b, v_w, v_b, out_w):
    f = np.float32
    pnp = _np_dt(PROJ_DT)
    onp = _np_dt(OUT_DT)
    maps = []
    for c in range(8):
        b, g = divmod(c, 2)
        fs = slice(g * FPC, (g + 1) * FPC)
        maps.append({
            "xt": np.ascontiguousarray(x[b].T).astype(pnp),
            "wqt": np.ascontiguousarray(q_w[fs, :].T).astype(pnp),
            "wkt": np.ascontiguousarray(k_w[fs, :].T).astype(pnp),
            "wvt": np.ascontiguousarray(v_w[fs, :].T).astype(pnp),
            "wot": np.ascontiguousarray(out_w[:, fs].T).astype(onp),
            "qb": np.ascontiguousarray(
                (q_b[fs].astype(f) / 8.0).reshape(FT, P).T),
            "kb": np.ascontiguousarray(k_b[fs].astype(f).reshape(FT, P).T),
            "vb": np.ascontiguousarray(
                v_b[fs].astype(f).reshape(1, FPC)).astype(pnp),
            "negb": np.ascontiguousarray(
                np.where(mask[b] == 0, f(-30000.0), f(0.0)).astype(f)
                .reshape(ST, P).T),
            "sel": _sel_const(),
        })
    return maps


def kernel(x, mask, q_w, q_b, k_w, k_b, v_w, v_b, out_w, out_b):
    global LAST_RESULT
    from concourse import bass_utils

    x = np.asarray(x, np.float32)
    mask = np.asarray(mask)
    nc = _get_nc()
    maps = _in_maps(x, mask, np.asarray(q_w, np.float32),
                    np.asarray(q_b, np.float32), np.asarray(k_w, np.float32),
                    np.asarray(k_b, np.float32), np.asarray(v_w, np.float32),
                    np.asarray(v_b, np.float32), np.asarray(out_w, np.float32))
    res = bass_utils.run_bass_kernel_spmd(
        nc, maps, core_ids=list(range(8)), trace=TRACE,
        trace_kwargs=TRACE_KWARGS)
    LAST_RESULT = res
    out_b = np.asarray(out_b, np.float32)
    full = np.empty((BS, SLEN, DIM), np.float32)
    for b in range(BS):
        full[b] = res.results[2 * b]["out"] + res.results[2 * b + 1]["out"] + out_b
    return full

